# revision 6
# baseline (speedup 1.0000x reference)
"""Trainium2 Bass kernel for nn_ConvAttnPoolPlusGram.

Model: word/concept embeddings -> concept-parent attention -> recombine ->
masked replace -> conv1d+tanh -> CAML label-wise attention pooling.

Sharding: data-parallel over batch B=8 across the 8 NeuronCores (one batch
element per core). Each core computes its full [Y, L] attention slab.

Key algebraic restructuring (all exact):
  - softmax over L skips the max-subtraction (logits are ~N(0, 0.03): exp is
    perfectly conditioned), so  att = exp(logits) / Z  with Z from the Scalar
    engine's fused activation accumulator.
  - y[b, :] = sum_f final_W * (att @ h)  is computed as
        sum_l p[y, l] * G[y, l] / Z[y],  G = final_W @ h^T
    i.e. a second Y x L matmul plus a fused multiply-rowreduce
    (tensor_tensor_reduce), avoiding any transposition of the big p tensor.
"""

import os
import sys

import numpy as np

for _p in ("/opt/trn_rl_repo",):
    if _p not in sys.path and os.path.isdir(_p):
        sys.path.append(_p)

import concourse.bass as bass
import concourse.mybir as mybir
import concourse.tile as tile
from concourse import bacc
from concourse.bass import IndirectOffsetOnAxis
from concourse.bass_utils import run_bass_kernel_spmd
from concourse.masks import make_identity

B, NTOK, E, K, NPAR = 8, 2500, 100, 60, 6
Y, F, KS, H = 8921, 50, 10, 20
L = NTOK + 1            # 2501
PADL = NTOK + KS        # 2510 (5 zeros each side)
PAD = KS // 2           # 5
NYT = (Y + 127) // 128  # 70 y-tiles
YPADDED = NYT * 128     # 8960
LCH = 512               # l-chunk for PSUM banks
NLC = (L + LCH - 1) // LCH  # 5
NXT = (NTOK + 127) // 128   # 20 token tiles
f32 = mybir.dt.float32
i32 = mybir.dt.int32
AF = mybir.ActivationFunctionType
OP = mybir.AluOpType

# att-scale engine: "gpsimd" | "vector" | "scalar" | "split"
SCALE_MODE = os.environ.get("KERNEL_SCALE_MODE", "split")
NYT_RUN = int(os.environ.get("KERNEL_NYT", str(NYT)))

_CACHE = {}


def _build():
    nc = bacc.Bacc()

    # ---- per-core DRAM I/O ----
    x_b = nc.dram_tensor("x_b", [NTOK, 1], i32, kind="ExternalInput")
    concepts_b = nc.dram_tensor("concepts_b", [1, K], i32, kind="ExternalInput")
    parents_b = nc.dram_tensor("parents_b", [K * NPAR, 1], i32, kind="ExternalInput")
    dm_b = nc.dram_tensor("dm_b", [K, 1], i32, kind="ExternalInput")
    embed_W = nc.dram_tensor("embed_W", [50002, E], f32, kind="ExternalInput")
    concept_W = nc.dram_tensor("concept_W", [1002, E], f32, kind="ExternalInput")
    fc1_W = nc.dram_tensor("fc1_W", [H, 2 * E], f32, kind="ExternalInput")
    fc1_b = nc.dram_tensor("fc1_b", [H, 1], f32, kind="ExternalInput")
    fc2_WT = nc.dram_tensor("fc2_WT", [H, 1], f32, kind="ExternalInput")
    fc2_b = nc.dram_tensor("fc2_b", [1, 1], f32, kind="ExternalInput")
    rec_W = nc.dram_tensor("rec_W", [E, 2 * E], f32, kind="ExternalInput")
    rec_b = nc.dram_tensor("rec_b", [E, 1], f32, kind="ExternalInput")
    conv_W2 = nc.dram_tensor("conv_W2", [F, E * KS], f32, kind="ExternalInput")
    conv_b = nc.dram_tensor("conv_b", [F, 1], f32, kind="ExternalInput")
    U_W = nc.dram_tensor("U_W", [Y, F], f32, kind="ExternalInput")
    final_W = nc.dram_tensor("final_W", [Y, F], f32, kind="ExternalInput")
    final_b = nc.dram_tensor("final_b", [Y, 1], f32, kind="ExternalInput")

    att_out = nc.dram_tensor("att_out", [Y, L], f32, kind="ExternalOutput")
    yhat_out = nc.dram_tensor("yhat_out", [YPADDED, 1], f32, kind="ExternalOutput")

    with tile.TileContext(nc) as tc:
        with (
            tc.tile_pool(name="singles", bufs=1) as singles,
            tc.tile_pool(name="gath", bufs=4) as gath,
            tc.tile_pool(name="lhs", bufs=3) as lhs,
            tc.tile_pool(name="prow", bufs=3) as prowp,
            tc.tile_pool(name="smalls", bufs=6) as smalls,
            tc.tile_pool(name="scr", bufs=2) as scr,
            tc.tile_pool(name="rowps", bufs=1, space="PSUM") as rowps,
            tc.tile_pool(name="gps", bufs=2, space="PSUM") as gps,
            tc.tile_pool(name="tps", bufs=1, space="PSUM") as tps,
        ):
            # ======== constants ========
            ident = singles.tile([128, 128], f32)
            make_identity(nc, ident[:])
            ones1 = singles.tile([1, 128], f32)
            nc.vector.memset(ones1[:], 1.0)

            # ======== load small weights ========
            fc1_sb = singles.tile([H, 2 * E], f32)
            nc.sync.dma_start(out=fc1_sb[:], in_=fc1_W[:])
            fc1b_sb = singles.tile([H, 1], f32)
            nc.sync.dma_start(out=fc1b_sb[:], in_=fc1_b[:])
            fc2T_sb = singles.tile([H, 1], f32)
            nc.sync.dma_start(out=fc2T_sb[:], in_=fc2_WT[:])
            fc2b_sb = singles.tile([1, 1], f32)
            nc.sync.dma_start(out=fc2b_sb[:], in_=fc2_b[:])
            rec_sb = singles.tile([E, 2 * E], f32)
            nc.sync.dma_start(out=rec_sb[:], in_=rec_W[:])
            recb_sb = singles.tile([E, 1], f32)
            nc.sync.dma_start(out=recb_sb[:], in_=rec_b[:])
            convw_sb = singles.tile([F, E * KS], f32)
            nc.sync.dma_start(out=convw_sb[:], in_=conv_W2[:])
            convb_sb = singles.tile([F, 1], f32)
            nc.sync.dma_start(out=convb_sb[:], in_=conv_b[:])
            conc_sb = singles.tile([1, K], i32)
            nc.sync.dma_start(out=conc_sb[:], in_=concepts_b[:])

            # U_W / final_W row-blocks (natural layout, transposed on PE later)
            U_sb = singles.tile([128, NYT * F], f32)
            fW_sb = singles.tile([128, NYT * F], f32)
            fb_all = singles.tile([128, NYT], f32)
            nc.vector.memset(fb_all[:], 0.0)
            for g in range(NYT):
                y0 = g * 128
                nt = min(128, Y - y0)
                nc.sync.dma_start(out=U_sb[:nt, g * F:(g + 1) * F],
                                  in_=U_W[y0:y0 + nt, :])
                nc.sync.dma_start(out=fW_sb[:nt, g * F:(g + 1) * F],
                                  in_=final_W[y0:y0 + nt, :])
                nc.sync.dma_start(out=fb_all[:nt, g:g + 1],
                                  in_=final_b[y0:y0 + nt, :])

            # ======== weight transposes (PE) ========
            def transpose_to(dst_ap, src_ap, n_rows, n_cols):
                """dst[ n_cols part, n_rows free ] = src[n_rows, n_cols].T"""
                tp = tps.tile([128, 128], f32, space="PSUM", tag="tp")
                nc.tensor.transpose(out=tp[:n_cols, :n_rows], in_=src_ap,
                                    identity=ident[:n_rows, :n_rows])
                nc.scalar.copy(out=dst_ap, in_=tp[:n_cols, :n_rows])

            W1aT = singles.tile([E, H], f32)
            transpose_to(W1aT[:], fc1_sb[:H, 0:E], H, E)
            W1bT = singles.tile([E, H], f32)
            transpose_to(W1bT[:], fc1_sb[:H, E:2 * E], H, E)
            recWaT = singles.tile([E, E], f32)
            transpose_to(recWaT[:], rec_sb[:E, 0:E], E, E)
            recWbT = singles.tile([E, E], f32)
            transpose_to(recWbT[:], rec_sb[:E, E:2 * E], E, E)
            # conv taps: WkT[e, f] = conv_W[f, e, k]
            convw_r = convw_sb[:].rearrange("f (e k) -> f e k", k=KS)
            WkT_all = singles.tile([E, KS * F], f32)
            for k in range(KS):
                transpose_to(WkT_all[:, k * F:(k + 1) * F], convw_r[:, :, k], F, E)

            # ======== token embedding gather -> xeT [E, PADL] ========
            xepad = singles.tile([E, PADL], f32)
            nc.vector.memset(xepad[:, 0:PAD], 0.0)
            nc.vector.memset(xepad[:, PAD + NTOK:PADL], 0.0)
            for t in range(NXT):
                t0 = t * 128
                nt = min(128, NTOK - t0)
                xi = gath.tile([128, 1], i32, tag="xidx")
                nc.sync.dma_start(out=xi[:nt, :], in_=x_b[t0:t0 + nt, :])
                xe_t = gath.tile([128, E], f32, tag="xe")
                nc.gpsimd.indirect_dma_start(
                    out=xe_t[:nt, :], out_offset=None, in_=embed_W[:],
                    in_offset=IndirectOffsetOnAxis(ap=xi[:nt, :1], axis=0))
                tp = tps.tile([128, 128], f32, space="PSUM", tag="tp")
                nc.tensor.transpose(out=tp[:E, :nt], in_=xe_t[:nt, :E],
                                    identity=ident[:nt, :nt])
                nc.scalar.copy(out=xepad[:, PAD + t0:PAD + t0 + nt], in_=tp[:E, :nt])

            # ======== concept-parent attention ========
            # gather parent embeddings -> peT [E, 360]
            peT = singles.tile([E, K * NPAR], f32)
            for t in range(3):
                t0 = t * 128
                nt = min(128, K * NPAR - t0)
                pi = gath.tile([128, 1], i32, tag="pidx")
                nc.sync.dma_start(out=pi[:nt, :], in_=parents_b[t0:t0 + nt, :])
                pe_t = gath.tile([128, E], f32, tag="pe")
                nc.gpsimd.indirect_dma_start(
                    out=pe_t[:nt, :], out_offset=None, in_=concept_W[:],
                    in_offset=IndirectOffsetOnAxis(ap=pi[:nt, :1], axis=0))
                tp = tps.tile([128, 128], f32, space="PSUM", tag="tp")
                nc.tensor.transpose(out=tp[:E, :nt], in_=pe_t[:nt, :E],
                                    identity=ident[:nt, :nt])
                nc.scalar.copy(out=peT[:, t0:t0 + nt], in_=tp[:E, :nt])

            # childrenT[:, k*6+j] = peT[:, k*6]
            chT = singles.tile([E, K * NPAR], f32)
            peT_r = peT[:].rearrange("p (k j) -> p k j", j=NPAR)
            chT_r = chT[:].rearrange("p (k j) -> p k j", j=NPAR)
            for j in range(NPAR):
                nc.vector.tensor_copy(out=chT_r[:, :, j], in_=peT_r[:, :, 0])

            # h1T = tanh(fc1 @ inptT + b): [H, 360]
            h1p = gps.tile([128, LCH], f32, space="PSUM", tag="gp")
            nc.tensor.matmul(out=h1p[:H, :K * NPAR], lhsT=W1aT[:, :H],
                             rhs=chT[:], start=True, stop=False)
            nc.tensor.matmul(out=h1p[:H, :K * NPAR], lhsT=W1bT[:, :H],
                             rhs=peT[:], start=False, stop=True)
            h1T = singles.tile([H, K * NPAR], f32)
            nc.scalar.activation(out=h1T[:], in_=h1p[:H, :K * NPAR], func=AF.Tanh,
                                 bias=fc1b_sb[:, :1], scale=1.0)

            # scores -> exp -> per-concept softmax pieces
            sp = gps.tile([128, LCH], f32, space="PSUM", tag="gp")
            nc.tensor.matmul(out=sp[:1, :K * NPAR], lhsT=fc2T_sb[:H, :1],
                             rhs=h1T[:], start=True, stop=True)
            es = singles.tile([1, K * NPAR], f32)
            nc.scalar.activation(out=es[:], in_=sp[:1, :K * NPAR], func=AF.Exp,
                                 bias=fc2b_sb[:1, :1], scale=1.0)
            z6 = singles.tile([1, K], f32)
            nc.vector.tensor_reduce(out=z6[:], in_=es[:].rearrange("p (k j) -> p k j", j=NPAR),
                                    axis=mybir.AxisListType.X, op=OP.add)
            r6 = singles.tile([1, K], f32)
            nc.vector.reciprocal(out=r6[:], in_=z6[:])

            # broadcast exp-scores to E partitions, weighted sum over parents
            esB = gps.tile([128, LCH], f32, space="PSUM", tag="gp")
            nc.tensor.matmul(out=esB[:E, :K * NPAR], lhsT=ones1[:1, :E],
                             rhs=es[:], start=True, stop=True)
            wp = singles.tile([E, K * NPAR], f32)
            nc.vector.tensor_mul(out=wp[:], in0=peT[:], in1=esB[:E, :K * NPAR])
            cnum = singles.tile([E, K], f32)
            nc.vector.tensor_reduce(out=cnum[:], in_=wp[:].rearrange("p (k j) -> p k j", j=NPAR),
                                    axis=mybir.AxisListType.X, op=OP.add)
            r6B = gps.tile([128, LCH], f32, space="PSUM", tag="gp")
            nc.tensor.matmul(out=r6B[:E, :K], lhsT=ones1[:1, :E],
                             rhs=r6[:], start=True, stop=True)
            cT = singles.tile([E, K], f32)
            nc.vector.tensor_mul(out=cT[:], in0=cnum[:], in1=r6B[:E, :K])

            # dm embedding -> dmeT [E, K]
            di = gath.tile([128, 1], i32, tag="didx")
            nc.sync.dma_start(out=di[:K, :], in_=dm_b[:, :])
            dme_t = gath.tile([128, E], f32, tag="dme")
            nc.gpsimd.indirect_dma_start(
                out=dme_t[:K, :], out_offset=None, in_=embed_W[:],
                in_offset=IndirectOffsetOnAxis(ap=di[:K, :1], axis=0))
            dmeT = singles.tile([E, K], f32)
            tpd = tps.tile([128, 128], f32, space="PSUM", tag="tp")
            nc.tensor.transpose(out=tpd[:E, :K], in_=dme_t[:K, :E],
                                identity=ident[:K, :K])
            nc.scalar.copy(out=dmeT[:], in_=tpd[:E, :K])

            # liT = rec_W @ cat(c, dme) + rec_b : [E, K]
            lp = gps.tile([128, LCH], f32, space="PSUM", tag="gp")
            nc.tensor.matmul(out=lp[:E, :K], lhsT=recWaT[:, :E], rhs=cT[:],
                             start=True, stop=False)
            nc.tensor.matmul(out=lp[:E, :K], lhsT=recWbT[:, :E], rhs=dmeT[:],
                             start=False, stop=True)
            liT = singles.tile([E, K], f32)
            nc.scalar.activation(out=liT[:], in_=lp[:E, :K], func=AF.Identity,
                                 bias=recb_sb[:, :1], scale=1.0)

            # masked replace of first K token columns where concepts > 0
            cmf = singles.tile([1, K], f32)
            nc.vector.tensor_scalar(out=cmf[:], in0=conc_sb[:], scalar1=0,
                                    scalar2=None, op0=OP.is_gt)
            mB = gps.tile([128, LCH], f32, space="PSUM", tag="gp")
            nc.tensor.matmul(out=mB[:E, :K], lhsT=ones1[:1, :E], rhs=cmf[:],
                             start=True, stop=True)
            mBi = singles.tile([E, K], i32)
            nc.vector.tensor_copy(out=mBi[:], in_=mB[:E, :K])
            nc.vector.copy_predicated(out=xepad[:, PAD:PAD + K],
                                      mask=mBi[:], data=liT[:])

            # ======== conv1d + tanh -> h [F, L] ========
            h_sb = singles.tile([F, L], f32)
            for c in range(NLC):
                c0 = c * LCH
                cw = min(LCH, L - c0)
                hp = gps.tile([128, LCH], f32, space="PSUM", tag="gp")
                for k in range(KS):
                    nc.tensor.matmul(out=hp[:F, :cw],
                                     lhsT=WkT_all[:, k * F:(k + 1) * F],
                                     rhs=xepad[:, c0 + k:c0 + k + cw],
                                     start=(k == 0), stop=(k == KS - 1))
                nc.scalar.activation(out=h_sb[:, c0:c0 + cw], in_=hp[:F, :cw],
                                     func=AF.Tanh, bias=convb_sb[:, :1], scale=1.0)

            # ======== per-label pooling state ========
            zall = singles.tile([128, NYT], f32)
            nc.vector.memset(zall[:], 1.0)
            dall = singles.tile([128, NYT], f32)
            nc.vector.memset(dall[:], 0.0)

            # ======== main loop over y-tiles ========
            for g in range(NYT_RUN):
                y0 = g * 128
                nt = min(128, Y - y0)

                UT = lhs.tile([F, 128], f32, tag="UT")
                tpu = tps.tile([128, 128], f32, space="PSUM", tag="tp")
                nc.tensor.transpose(out=tpu[:F, :nt], in_=U_sb[:nt, g * F:(g + 1) * F],
                                    identity=ident[:nt, :nt])
                nc.scalar.copy(out=UT[:, :nt], in_=tpu[:F, :nt])
                FT = lhs.tile([F, 128], f32, tag="FT")
                tpf = tps.tile([128, 128], f32, space="PSUM", tag="tp")
                nc.tensor.transpose(out=tpf[:F, :nt], in_=fW_sb[:nt, g * F:(g + 1) * F],
                                    identity=ident[:nt, :nt])
                nc.scalar.copy(out=FT[:, :nt], in_=tpf[:F, :nt])

                # logits row [nt, L] in PSUM (5 banks)
                rowp = rowps.tile([128, L], f32, space="PSUM")
                for c in range(NLC):
                    c0 = c * LCH
                    cw = min(LCH, L - c0)
                    nc.tensor.matmul(out=rowp[:nt, c0:c0 + cw], lhsT=UT[:, :nt],
                                     rhs=h_sb[:, c0:c0 + cw], start=True, stop=True)

                # p = exp(logits) with fused row-sum Z
                p_row = prowp.tile([128, L], f32)
                nc.scalar.activation(out=p_row[:nt, :], in_=rowp[:nt, :],
                                     func=AF.Exp, accum_out=zall[:nt, g:g + 1])

                # G row chunks + dot accumulation.
                # (tensor_tensor_reduce faults the exec unit on this runtime,
                # so: VectorE multiply, then row-sum split between ScalarE's
                # activation accumulator and VectorE tensor_reduce.)
                dacc = smalls.tile([128, NLC], f32, tag="dacc")
                for c in range(NLC):
                    c0 = c * LCH
                    cw = min(LCH, L - c0)
                    gp = gps.tile([128, LCH], f32, space="PSUM", tag="gp")
                    nc.tensor.matmul(out=gp[:nt, :cw], lhsT=FT[:, :nt],
                                     rhs=h_sb[:, c0:c0 + cw], start=True, stop=True)
                    sc = scr.tile([128, LCH], f32, tag="sc")
                    nc.vector.tensor_mul(out=sc[:nt, :cw],
                                         in0=p_row[:nt, c0:c0 + cw],
                                         in1=gp[:nt, :cw])
                    if c < 3:
                        junk = scr.tile([128, LCH], f32, tag="junk")
                        nc.scalar.activation(out=junk[:nt, :cw], in_=sc[:nt, :cw],
                                             func=AF.Identity,
                                             accum_out=dacc[:nt, c:c + 1])
                    else:
                        nc.vector.tensor_reduce(out=dacc[:nt, c:c + 1],
                                                in_=sc[:nt, :cw],
                                                axis=mybir.AxisListType.X, op=OP.add)
                nc.vector.tensor_reduce(out=dall[:nt, g:g + 1],
                                        in_=dacc[:nt, :NLC],
                                        axis=mybir.AxisListType.X, op=OP.add)

                # att = p / Z
                rz = smalls.tile([128, 1], f32, tag="rz")
                nc.vector.reciprocal(out=rz[:nt, :], in_=zall[:nt, g:g + 1])
                if SCALE_MODE == "gpsimd":
                    nc.gpsimd.tensor_scalar(out=p_row[:nt, :], in0=p_row[:nt, :],
                                            scalar1=rz[:nt, :1], scalar2=None,
                                            op0=OP.mult)
                elif SCALE_MODE == "vector":
                    nc.vector.tensor_scalar(out=p_row[:nt, :NTOK], in0=p_row[:nt, :NTOK],
                                            scalar1=rz[:nt, :1], scalar2=None, op0=OP.mult)
                    nc.vector.tensor_scalar(out=p_row[:nt, NTOK:L], in0=p_row[:nt, NTOK:L],
                                            scalar1=rz[:nt, :1], scalar2=None, op0=OP.mult)
                elif SCALE_MODE == "scalar":
                    nc.scalar.mul(p_row[:nt, :], p_row[:nt, :], rz[:nt, :1])
                else:  # split
                    if g % 2 == 0:
                        nc.scalar.mul(p_row[:nt, :], p_row[:nt, :], rz[:nt, :1])
                    else:
                        nc.vector.tensor_scalar(out=p_row[:nt, :NTOK], in0=p_row[:nt, :NTOK],
                                                scalar1=rz[:nt, :1], scalar2=None, op0=OP.mult)
                        nc.vector.tensor_scalar(out=p_row[:nt, NTOK:L], in0=p_row[:nt, NTOK:L],
                                                scalar1=rz[:nt, :1], scalar2=None, op0=OP.mult)

                nc.sync.dma_start(out=att_out[y0:y0 + nt, :], in_=p_row[:nt, :])

            # ======== yhat epilogue (batched over [128, NYT]) ========
            rzall = singles.tile([128, NYT], f32)
            nc.vector.reciprocal(out=rzall[:], in_=zall[:])
            yv = singles.tile([128, NYT], f32)
            nc.vector.tensor_mul(out=yv[:], in0=dall[:], in1=rzall[:])
            ev = singles.tile([128, NYT], f32)
            nc.scalar.activation(out=ev[:], in_=yv[:], func=AF.Exp, scale=-1.0)
            efb = singles.tile([128, NYT], f32)
            nc.scalar.activation(out=efb[:], in_=fb_all[:], func=AF.Exp, scale=-1.0)
            e2 = singles.tile([128, NYT], f32)
            nc.vector.tensor_mul(out=e2[:], in0=ev[:], in1=efb[:])
            nc.vector.tensor_scalar(out=e2[:], in0=e2[:], scalar1=1.0, scalar2=None,
                                    op0=OP.add)
            yh = singles.tile([128, NYT], f32)
            nc.vector.reciprocal(out=yh[:], in_=e2[:])
            # transpose to [NYT, 128] so the DRAM write is contiguous
            tpy = tps.tile([128, 128], f32, space="PSUM", tag="tp")
            nc.tensor.transpose(out=tpy[:NYT, :128], in_=yh[:, :NYT],
                                identity=ident[:, :])
            yhT = singles.tile([NYT, 128], f32)
            nc.scalar.copy(out=yhT[:], in_=tpy[:NYT, :128])
            nc.sync.dma_start(
                out=yhat_out[:].rearrange("(g p) o -> g (p o)", p=128),
                in_=yhT[:])

    nc.finalize()
    return nc


def _get_nc():
    if "nc" not in _CACHE:
        _CACHE["nc"] = _build()
    return _CACHE["nc"]


def _make_in_maps(inputs):
    f = lambda a: np.ascontiguousarray(np.asarray(a), dtype=np.float32)
    ii = lambda a: np.ascontiguousarray(np.asarray(a), dtype=np.int32)
    x = ii(inputs["x"])
    concepts = ii(inputs["concepts"])
    parents = ii(inputs["parents"])
    dm = ii(inputs["dm"])
    shared = dict(
        embed_W=f(inputs["embed_W"]),
        concept_W=f(inputs["concept_W"]),
        fc1_W=f(inputs["fc1_W"]),
        fc1_b=f(inputs["fc1_b"]).reshape(H, 1),
        fc2_WT=f(inputs["fc2_W"]).reshape(1, H).T.copy(),
        fc2_b=f(inputs["fc2_b"]).reshape(1, 1),
        rec_W=f(inputs["rec_W"]),
        rec_b=f(inputs["rec_b"]).reshape(E, 1),
        conv_W2=f(inputs["conv_W"]).reshape(F, E * KS),
        conv_b=f(inputs["conv_b"]).reshape(F, 1),
        U_W=f(inputs["U_W"]),
        final_W=f(inputs["final_W"]),
        final_b=f(inputs["final_b"]).reshape(Y, 1),
    )
    in_maps = []
    for b in range(B):
        m = dict(shared)
        m["x_b"] = x[b].reshape(NTOK, 1)
        m["concepts_b"] = concepts[b, :K].reshape(1, K)
        m["parents_b"] = parents[b].reshape(K * NPAR, 1)
        m["dm_b"] = dm[b].reshape(K, 1)
        in_maps.append(m)
    return in_maps


def _loss_host(yhat, target):
    p = np.clip(yhat, np.float32(1e-7), np.float32(1.0 - 1e-7)).astype(np.float32)
    t = np.asarray(target, dtype=np.float32)
    ll = t * np.log(p) + (1.0 - t) * np.log1p(-p)
    return np.float32(-np.mean(ll))


def run(inputs, trace=False):
    """Returns ((yhat, loss, att), exec_time_ns)."""
    nc = _get_nc()
    in_maps = _make_in_maps(inputs)
    res = run_bass_kernel_spmd(nc, in_maps, core_ids=list(range(B)), trace=trace)
    att = np.stack([res.results[b]["att_out"] for b in range(B)], axis=0)
    yhat = np.stack([res.results[b]["yhat_out"][:Y, 0] for b in range(B)], axis=0)
    loss = _loss_host(yhat, inputs["target"])
    return (yhat, loss, att), res.exec_time_ns


def kernel(**inputs):
    out, _ = run(inputs, trace=False)
    return out
